# revision 11
# baseline (speedup 1.0000x reference)
"""Causal self-attention with KV cache + per-head distance-decay bias, on 8 trn2 cores.

Sharding: tensor-parallel over heads (16 heads -> 2 per core). Each core:
  - projects full x with its shard of W_attn (q/k transposed layout, v natural)
  - applies RoPE to q/k, concatenates cached KV
  - computes scores^T [keys, queries] tiles, adds a Toeplitz bias band
    (causal mask + -log1p(decay*log1p(dist))), exp, attention @ V
  - projects with its shard of W_proj rows -> partial y
Host sums the 8 partial y outputs and assembles the full k/v caches.
"""

import numpy as np
import ml_dtypes
from contextlib import ExitStack

import concourse.bass as bass
import concourse.mybir as mybir
import concourse.tile as tile
from concourse import bacc
from concourse.bass_utils import run_bass_kernel_spmd

F32 = mybir.dt.float32
F32R = mybir.dt.float32r
BF16 = mybir.dt.bfloat16
AF = mybir.ActivationFunctionType
ALU = mybir.AluOpType

H, T, OFF, C, HS = 16, 1024, 1024, 2048, 128
NCORES, HL = 8, 2          # heads per core
TF = OFF + T               # 2048 full key length
KC = C // 128              # 16 contraction chunks
NT = T // 128              # 8 t-chunks
NSL = T // 512             # 2 t-slabs
NU = TF // 128             # 16 u-chunks
BW = 2432                  # bias band width: (c_max - c_min) + 512 = 1536+384+512
CMIN = -384
MASKVAL = -10000.0


def _nu_of_slab(sl):
    # chunks uc with c = sl*512 + 1024 - uc*128 >= -384
    return 12 if sl == 0 else 16


def _build_nc():
    nc = bacc.Bacc(
        "TRN2", target_bir_lowering=False, debug=False, enable_asserts=False
    )

    xT = nc.dram_tensor("xT", [KC, 128, T], F32R, kind="ExternalInput")
    wqk = nc.dram_tensor("wqk", [KC, 128, 4 * 128], F32R, kind="ExternalInput")
    wv = nc.dram_tensor("wv", [KC, 128, HL * 128], F32R, kind="ExternalInput")
    bqk = nc.dram_tensor("bqk", [128, 4], F32, kind="ExternalInput")
    bv = nc.dram_tensor("bv", [128, HL * 128], F32, kind="ExternalInput")
    cosf = nc.dram_tensor("cosf", [128, T], F32, kind="ExternalInput")
    sinf = nc.dram_tensor("sinf", [128, T], F32, kind="ExternalInput")
    wp = nc.dram_tensor("wp", [HL, 128, C], F32R, kind="ExternalInput")
    ckT = nc.dram_tensor("ckT", [HL, 128, OFF], F32R, kind="ExternalInput")
    cv = nc.dram_tensor("cv", [HL, OFF // 128, 128, HS], BF16, kind="ExternalInput")
    wband_in = nc.dram_tensor("wband_in", [HL, 128, BW], F32, kind="ExternalInput")

    y_out = nc.dram_tensor("y_out", [T, C], F32, kind="ExternalOutput")
    kT_new = nc.dram_tensor("kT_new", [HL, 128, T], F32R, kind="ExternalOutput")
    v_new = nc.dram_tensor("v_new", [NT, 128, HL * 128], F32, kind="ExternalOutput")

    with tile.TileContext(nc) as tc, ExitStack() as ctx:
        const = ctx.enter_context(tc.tile_pool(name="const", bufs=1))
        state = ctx.enter_context(tc.tile_pool(name="state", bufs=1))
        xt_pool = ctx.enter_context(tc.tile_pool(name="xt", bufs=KC))
        wtile = ctx.enter_context(tc.tile_pool(name="wtile", bufs=3))
        tmp = ctx.enter_context(tc.tile_pool(name="tmp", bufs=2))
        sbtp = ctx.enter_context(tc.tile_pool(name="sbtp", bufs=4))
        ppool = ctx.enter_context(tc.tile_pool(name="ppool", bufs=6))

        # ---- constants ----
        cos_sb = const.tile([128, T], F32)
        sin_sb = const.tile([128, T], F32)
        bqk_sb = const.tile([128, 4], F32)
        bv_sb = const.tile([128, HL * 128], F32)
        ones_sb = const.tile([128, 128], BF16)
        wp_sb = const.tile([128, HL, C], F32R)
        nc.sync.dma_start(out=cos_sb, in_=cosf[:, :])
        nc.sync.dma_start(out=sin_sb, in_=sinf[:, :])
        nc.sync.dma_start(out=bqk_sb, in_=bqk[:, :])
        nc.sync.dma_start(out=bv_sb, in_=bv[:, :])
        nc.vector.memset(ones_sb, 1.0)
        nc.sync.dma_start(
            out=wp_sb, in_=wp[:, :, :].rearrange("h p c -> p h c")
        )

        # ---- persistent activations ----
        kT_sb = state.tile([128, HL, TF], F32R)   # [hs, head, keys]
        qT_sb = state.tile([128, HL, T], F32R)    # [hs, head, queries]
        v_sb = state.tile([128, HL, NU, HS], BF16)  # [t%128, head, uc, hs]
        yT_sb = state.tile([128, HL, T], F32R)    # [hs, head, queries]
        wband = state.tile([128, HL, BW], F32)   # Toeplitz bias band per head

        for h in range(HL):
            nc.sync.dma_start(out=kT_sb[:, h, 0:OFF], in_=ckT[h, :, :])
            nc.sync.dma_start(
                out=v_sb[:, h, 0 : OFF // 128, :],
                in_=cv[h, :, :, :].rearrange("u p d -> p u d"),
            )

        # ---- bias band (host-computed Toeplitz causal+decay bias) ----
        nc.sync.dma_start(
            out=wband, in_=wband_in[:, :, :].rearrange("h p i -> p h i")
        )

        # ---- load xT resident ----
        xt = []
        for kc in range(KC):
            t_ = xt_pool.tile([128, T], F32R, tag="xt")
            nc.sync.dma_start(out=t_, in_=xT[kc, :, :])
            xt.append(t_)

        # ---- phase 1: q/k projection (transposed out) + RoPE ----
        # chunk order: 0=q_h0, 1=q_h1, 2=k_h0, 3=k_h1 (q cols pre-scaled on host)
        with tc.tile_pool(name="pqk", bufs=8, space="PSUM") as pqk:
            qkp = [
                [pqk.tile([128, 512], F32, tag="qkp", name="qkp") for _ in range(NSL)]
                for _ in range(4)
            ]
            for kc in range(KC):
                wt = wtile.tile([128, 4 * 128], F32R, tag="wqk")
                nc.gpsimd.dma_start(out=wt, in_=wqk[kc, :, :])
                for ch in range(4):
                    for sl in range(NSL):
                        nc.tensor.matmul(
                            qkp[ch][sl],
                            wt[:, ch * 128 : (ch + 1) * 128],
                            xt[kc][:, sl * 512 : (sl + 1) * 512],
                            start=(kc == 0),
                            stop=(kc == KC - 1),
                        )
            for ch in range(4):
                h = ch % 2
                is_q = ch < 2
                for sl in range(NSL):
                    ts_ = slice(sl * 512, (sl + 1) * 512)
                    if is_q:
                        dest = qT_sb[:, h, ts_]
                    else:
                        dest = kT_sb[:, h, OFF + sl * 512 : OFF + (sl + 1) * 512]
                    ps = qkp[ch][sl]
                    bcol = bqk_sb[:, ch : ch + 1]
                    m1 = tmp.tile([128, 512], F32, tag="ropem1")
                    t2 = tmp.tile([128, 512], F32, tag="ropet2")
                    nc.vector.scalar_tensor_tensor(
                        m1, ps, bcol, cos_sb[:, ts_], ALU.add, ALU.mult
                    )
                    nc.vector.scalar_tensor_tensor(
                        t2[0:64],
                        ps[64:128],
                        bqk_sb[64:128, ch : ch + 1],
                        sin_sb[64:128, ts_],
                        ALU.add,
                        ALU.mult,
                    )
                    nc.vector.scalar_tensor_tensor(
                        t2[64:128],
                        ps[0:64],
                        bqk_sb[0:64, ch : ch + 1],
                        sin_sb[0:64, ts_],
                        ALU.add,
                        ALU.mult,
                    )
                    nc.vector.tensor_add(dest, m1, t2)

        # ---- phase 1b: v projection (natural layout) ----
        with tc.tile_pool(name="pv", bufs=8, space="PSUM") as pv:
            vps = [pv.tile([128, HL * 128], F32, tag="vp", name="vp") for _ in range(NT)]
            for kc in range(KC):
                wt = wtile.tile([128, HL * 128], F32R, tag="wv")
                nc.gpsimd.dma_start(out=wt, in_=wv[kc, :, :])
                for tc8 in range(NT):
                    nc.tensor.matmul(
                        vps[tc8],
                        xt[kc][:, tc8 * 128 : (tc8 + 1) * 128],
                        wt,
                        start=(kc == 0),
                        stop=(kc == KC - 1),
                    )
            for tc8 in range(NT):
                vf = tmp.tile([128, HL * 128], F32, tag="vf")
                nc.vector.tensor_add(vf, vps[tc8], bv_sb)
                nc.sync.dma_start(out=v_new[tc8, :, :], in_=vf)
                for h in range(HL):
                    nc.vector.tensor_copy(
                        out=v_sb[:, h, OFF // 128 + tc8, :],
                        in_=vf[:, h * 128 : (h + 1) * 128],
                    )

        # ---- phase 2: attention, scores^T layout [u, t] ----
        PIPE = 3
        with (
            tc.tile_pool(name="psc", bufs=4, space="PSUM") as psc,
            tc.tile_pool(name="pyt", bufs=2, space="PSUM") as pyt,
            tc.tile_pool(name="pss", bufs=2, space="PSUM") as pss,
        ):
            for h in range(HL):
                for sl in range(NSL):
                    nu = _nu_of_slab(sl)
                    ts_ = slice(sl * 512, (sl + 1) * 512)
                    ytp = pyt.tile([128, 512], F32, tag="ytp")
                    ssp = pss.tile([128, 512], F32, tag="ssp")
                    pts = [None] * nu

                    def consume(uc):
                        nc.tensor.matmul(
                            ytp,
                            v_sb[:, h, uc, :],
                            pts[uc],
                            start=(uc == 0),
                            stop=(uc == nu - 1),
                        )
                        nc.tensor.matmul(
                            ssp,
                            ones_sb,
                            pts[uc],
                            start=(uc == 0),
                            stop=(uc == nu - 1),
                        )

                    for uc in range(nu):
                        scp = psc.tile([128, 512], F32, tag="scp")
                        nc.tensor.matmul(
                            scp,
                            kT_sb[:, h, uc * 128 : (uc + 1) * 128],
                            qT_sb[:, h, ts_],
                            start=True,
                            stop=True,
                        )
                        off = sl * 512 + 1408 - uc * 128
                        sbt = sbtp.tile([128, 512], F32, tag="sbt")
                        nc.vector.tensor_add(
                            sbt, scp, wband[:, h, off : off + 512]
                        )
                        pt = ppool.tile([128, 512], BF16, tag="pt")
                        nc.scalar.activation(out=pt, in_=sbt, func=AF.Exp)
                        pts[uc] = pt
                        if uc >= PIPE:
                            consume(uc - PIPE)
                    for uc in range(max(0, nu - PIPE), nu):
                        consume(uc)

                    inv = tmp.tile([128, 512], F32, tag="inv")
                    nc.vector.reciprocal(out=inv, in_=ssp)
                    nc.vector.tensor_mul(yT_sb[:, h, ts_], ytp, inv)

        # ---- phase 3: output projection (partial y) ----
        with tc.tile_pool(name="po", bufs=4, space="PSUM") as po:
            for tc8 in range(NT):
                for ns in range(4):
                    pop = po.tile([128, 512], F32, tag="pop")
                    for h in range(HL):
                        nc.tensor.matmul(
                            pop,
                            yT_sb[:, h, tc8 * 128 : (tc8 + 1) * 128],
                            wp_sb[:, h, ns * 512 : (ns + 1) * 512],
                            start=(h == 0),
                            stop=(h == HL - 1),
                        )
                    ystg = tmp.tile([128, 512], F32, tag="ystg")
                    nc.vector.tensor_copy(out=ystg, in_=pop)
                    nc.sync.dma_start(
                        out=y_out[
                            tc8 * 128 : (tc8 + 1) * 128, ns * 512 : (ns + 1) * 512
                        ],
                        in_=ystg,
                    )

        # ---- phase 4: new-k output ----
        for h in range(HL):
            nc.sync.dma_start(out=kT_new[h, :, :], in_=kT_sb[:, h, OFF:TF])

    nc.compile()
    return nc


def kernel(x, cached_k, cached_v, W_attn, b_attn, W_proj, b_proj, decay_raw):
    x = np.asarray(x, np.float32)
    cached_k = np.asarray(cached_k, np.float32)
    cached_v = np.asarray(cached_v, np.float32)
    W_attn = np.asarray(W_attn, np.float32)
    b_attn = np.asarray(b_attn, np.float32)
    W_proj = np.asarray(W_proj, np.float32)
    b_proj = np.asarray(b_proj, np.float32)
    decay_raw = np.asarray(decay_raw, np.float32)

    scale = np.float32(1.0 / np.sqrt(HS))
    xT = np.ascontiguousarray(x[0].T).reshape(KC, 128, T)

    # RoPE tables (match reference fp32 computation)
    pos = np.arange(OFF, OFF + T).astype(np.float32)
    inv_freq = (
        np.float32(1.0)
        / (np.float32(10000.0) ** (np.arange(HS // 2, dtype=np.float32) / np.float32(HS // 2)))
    ).astype(np.float32)
    ang = (pos[:, None] * inv_freq[None, :]).astype(np.float32)  # [T, 64]
    cos = np.cos(ang).astype(np.float32).T  # [64, T]
    sin = np.sin(ang).astype(np.float32).T
    cosf = np.concatenate([cos, cos], axis=0)        # [128, T]
    sinf = np.concatenate([sin, -sin], axis=0)       # [128, T]

    decay = np.log1p(np.exp(decay_raw.astype(np.float64))).astype(np.float32)

    ii = np.arange(BW)[None, :] - np.arange(128)[:, None] + CMIN  # d = i - p - 384
    dpos = np.maximum(ii, 0).astype(np.float64)
    logd = np.log1p(dpos)  # [128, BW]

    nc = _build_nc()

    in_maps = []
    for c in range(NCORES):
        g0 = HL * c
        qcols = W_attn[:, g0 * HS : (g0 + HL) * HS] * scale
        kcols = W_attn[:, C + g0 * HS : C + (g0 + HL) * HS]
        vcols = W_attn[:, 2 * C + g0 * HS : 2 * C + (g0 + HL) * HS]
        wqk_c = np.ascontiguousarray(
            np.concatenate([qcols, kcols], axis=1)
        ).reshape(KC, 128, 4 * 128)
        wv_c = np.ascontiguousarray(vcols).reshape(KC, 128, HL * 128)

        bq = b_attn[g0 * HS : (g0 + HL) * HS] * scale
        bk = b_attn[C + g0 * HS : C + (g0 + HL) * HS]
        bqk_c = np.concatenate([bq, bk]).reshape(4, 128).T.copy()  # [128, 4]
        bv_c = np.broadcast_to(
            b_attn[2 * C + g0 * HS : 2 * C + (g0 + HL) * HS][None, :], (128, HL * 128)
        ).copy()

        wp_c = np.ascontiguousarray(
            W_proj[g0 * HS : (g0 + HL) * HS, :]
        ).reshape(HL, 128, C)
        ckT_c = np.ascontiguousarray(cached_k[0, g0 : g0 + HL].transpose(0, 2, 1))
        cv_c = np.ascontiguousarray(
            cached_v[0, g0 : g0 + HL].reshape(HL, OFF // 128, 128, HS)
        ).astype(ml_dtypes.bfloat16)
        wband_c = np.empty((HL, 128, BW), np.float32)
        for l in range(HL):
            val = -np.log1p(np.float64(decay[g0 + l]) * logd)
            wband_c[l] = np.where(ii >= 0, val, MASKVAL).astype(np.float32)

        in_maps.append(
            {
                "xT": xT,
                "wqk": wqk_c,
                "wv": wv_c,
                "bqk": bqk_c,
                "bv": bv_c,
                "cosf": cosf,
                "sinf": sinf,
                "wp": wp_c,
                "ckT": ckT_c,
                "cv": cv_c,
                "wband_in": wband_c,
            }
        )

    res = run_bass_kernel_spmd(nc, in_maps, core_ids=list(range(NCORES)))
    results = res.results

    # ---- gather ----
    y = np.zeros((T, C), np.float64)
    for c in range(NCORES):
        y += results[c]["y_out"].astype(np.float64)
    y = (y.astype(np.float32) + b_proj[None, :]).reshape(1, T, C)

    k_full = np.empty((1, H, TF, HS), np.float32)
    v_full = np.empty((1, H, TF, HS), np.float32)
    k_full[0, :, :OFF] = cached_k[0]
    v_full[0, :, :OFF] = cached_v[0]
    for c in range(NCORES):
        kT_n = results[c]["kT_new"]  # [HL, 128, T]
        v_n = results[c]["v_new"]    # [NT, 128, HL*128]
        for l in range(HL):
            g = HL * c + l
            k_full[0, g, OFF:] = kT_n[l].T
            v_full[0, g, OFF:] = v_n[:, :, l * 128 : (l + 1) * 128].reshape(T, HS)

    return (y, k_full, v_full)


# revision 15
# speedup vs baseline: 1.0930x; 1.0930x over previous
"""Causal self-attention with KV cache + per-head distance-decay bias, on 8 trn2 cores.

Sharding: tensor-parallel over heads (16 heads -> 2 per core). Each core:
  - projects full x with its shard of W_attn (fp16 matmuls, all outputs transposed
    [feature, token]), applies RoPE to q/k (scale 1/sqrt(hs) folded into q's
    cos/sin tables), PE-transposes v back to [token, feature] in bf16
  - computes scores^T [keys, queries] tiles in fp32r, adds a host-precomputed
    Toeplitz bias band (causal mask + -log1p(decay*log1p(dist))), exp -> bf16,
    attention @ V in bf16 (denominators via an all-ones matmul)
  - projects with its shard of W_proj rows (bf16) -> partial y
Host sums the 8 partial y outputs and assembles the full k/v caches.
"""

import numpy as np
import ml_dtypes
from contextlib import ExitStack

import concourse.bass as bass
import concourse.mybir as mybir
import concourse.tile as tile
from concourse import bacc
from concourse.masks import make_identity
from concourse.bass_utils import run_bass_kernel_spmd

F32 = mybir.dt.float32
F32R = mybir.dt.float32r
F16 = mybir.dt.float16
BF16 = mybir.dt.bfloat16
AF = mybir.ActivationFunctionType
ALU = mybir.AluOpType

H, T, OFF, C, HS = 16, 1024, 1024, 2048, 128
NCORES, HL = 8, 2          # heads per core
TF = OFF + T               # 2048 full key length
KC = C // 128              # 16 contraction chunks
NT = T // 128              # 8 t-chunks
NSL = T // 512             # 2 t-slabs
NU = TF // 128             # 16 u-chunks
BW = 2432                  # bias band width: (c_max - c_min) + 512
CMIN = -384
MASKVAL = -10000.0


def _nu_of_slab(sl):
    # u-chunks uc with c = sl*512 + 1024 - uc*128 >= -384
    return 12 if sl == 0 else 16


def _build_nc():
    nc = bacc.Bacc(
        "TRN2", target_bir_lowering=False, debug=False, enable_asserts=False
    )

    xT = nc.dram_tensor("xT", [KC, 128, T], F16, kind="ExternalInput")
    wqkv = nc.dram_tensor("wqkv", [KC, 128, 6 * 128], F16, kind="ExternalInput")
    bqkv = nc.dram_tensor("bqkv", [128, 6], F32, kind="ExternalInput")
    cosf = nc.dram_tensor("cosf", [128, 2, T], F32, kind="ExternalInput")
    sinf = nc.dram_tensor("sinf", [128, 2, T], F32, kind="ExternalInput")
    wp = nc.dram_tensor("wp", [HL, 128, C], BF16, kind="ExternalInput")
    ckT = nc.dram_tensor("ckT", [HL, 128, OFF], F32R, kind="ExternalInput")
    cv = nc.dram_tensor("cv", [HL, OFF // 128, 128, HS], BF16, kind="ExternalInput")
    wband_in = nc.dram_tensor("wband_in", [HL, 128, BW], F32, kind="ExternalInput")

    y_out = nc.dram_tensor("y_out", [T, C], F32, kind="ExternalOutput")
    kT_new = nc.dram_tensor("kT_new", [HL, 128, T], F32R, kind="ExternalOutput")
    v_newT = nc.dram_tensor("v_newT", [HL, 128, T], F32, kind="ExternalOutput")

    with tile.TileContext(nc) as tc, ExitStack() as ctx:
        const = ctx.enter_context(tc.tile_pool(name="const", bufs=1))
        state = ctx.enter_context(tc.tile_pool(name="state", bufs=1))
        xt_pool = ctx.enter_context(tc.tile_pool(name="xt", bufs=KC))
        wtile = ctx.enter_context(tc.tile_pool(name="wtile", bufs=3))
        tmp = ctx.enter_context(tc.tile_pool(name="tmp", bufs=2))
        sbtp = ctx.enter_context(tc.tile_pool(name="sbtp", bufs=4))
        ppool = ctx.enter_context(tc.tile_pool(name="ppool", bufs=6))

        # ---- constants ----
        cos_sb = const.tile([128, 2, T], F32)   # [:, 0, :] q-scaled, [:, 1, :] k
        sin_sb = const.tile([128, 2, T], F32)
        bqkv_sb = const.tile([128, 6], F32)
        ones_sb = const.tile([128, 128], BF16)
        ident_sb = const.tile([128, 128], F32)
        wp_sb = const.tile([128, HL, C], BF16)
        nc.sync.dma_start(out=cos_sb, in_=cosf[:, :, :])
        nc.sync.dma_start(out=sin_sb, in_=sinf[:, :, :])
        nc.sync.dma_start(out=bqkv_sb, in_=bqkv[:, :])
        nc.vector.memset(ones_sb, 1.0)
        make_identity(nc, ident_sb)
        nc.sync.dma_start(
            out=wp_sb, in_=wp[:, :, :].rearrange("h p c -> p h c")
        )

        # ---- persistent activations ----
        kT_sb = state.tile([128, HL, TF], F32R)     # [hs, head, keys]
        qT_sb = state.tile([128, HL, T], F32R)      # [hs, head, queries]
        vT_sb = state.tile([128, HL, T], F32)       # [hs, head, new tokens]
        v_sb = state.tile([128, HL, NU, HS], BF16)  # [tok%128, head, uc, hs]
        yT_sb = state.tile([128, HL, T], BF16)      # [hs, head, queries]
        wband = state.tile([128, HL, BW], F32)      # Toeplitz bias band per head

        for h in range(HL):
            nc.sync.dma_start(out=kT_sb[:, h, 0:OFF], in_=ckT[h, :, :])
            nc.sync.dma_start(
                out=v_sb[:, h, 0 : OFF // 128, :],
                in_=cv[h, :, :, :].rearrange("u p d -> p u d"),
            )
        nc.sync.dma_start(
            out=wband, in_=wband_in[:, :, :].rearrange("h p i -> p h i")
        )

        # ---- load xT resident ----
        xt = []
        for kc in range(KC):
            t_ = xt_pool.tile([128, T], F16, tag="xt")
            nc.sync.dma_start(out=t_, in_=xT[kc, :, :])
            xt.append(t_)

        # ---- phase 1a: q/k projection (transposed out) + RoPE ----
        # chunk order in wqkv: 0=q_h0, 1=q_h1, 2=k_h0, 3=k_h1, 4=v_h0, 5=v_h1
        with tc.tile_pool(name="pqk", bufs=8, space="PSUM") as pqk:
            qkp = [
                [pqk.tile([128, 512], F32, tag="qkp", name="qkp") for _ in range(NSL)]
                for _ in range(4)
            ]
            for kc in range(KC):
                wt = wtile.tile([128, 512], F16, tag="wqk")
                nc.gpsimd.dma_start(out=wt, in_=wqkv[kc, :, 0:512])
                for ch in range(4):
                    for sl in range(NSL):
                        nc.tensor.matmul(
                            qkp[ch][sl],
                            wt[:, ch * 128 : (ch + 1) * 128],
                            xt[kc][:, sl * 512 : (sl + 1) * 512],
                            start=(kc == 0),
                            stop=(kc == KC - 1),
                        )
            for ch in range(4):
                h = ch % 2
                is_q = ch < 2
                tab = 0 if is_q else 1  # q tables carry the 1/sqrt(hs) scale
                for sl in range(NSL):
                    ts_ = slice(sl * 512, (sl + 1) * 512)
                    if is_q:
                        dest = qT_sb[:, h, ts_]
                    else:
                        dest = kT_sb[:, h, OFF + sl * 512 : OFF + (sl + 1) * 512]
                    ps = qkp[ch][sl]
                    bcol = bqkv_sb[:, ch : ch + 1]
                    m1 = tmp.tile([128, 512], F32, tag="ropem1")
                    t2 = tmp.tile([128, 512], F32, tag="ropet2")
                    nc.vector.scalar_tensor_tensor(
                        m1, ps, bcol, cos_sb[:, tab, ts_], ALU.add, ALU.mult
                    )
                    nc.vector.scalar_tensor_tensor(
                        t2[0:64],
                        ps[64:128],
                        bqkv_sb[64:128, ch : ch + 1],
                        sin_sb[64:128, tab, ts_],
                        ALU.add,
                        ALU.mult,
                    )
                    nc.vector.scalar_tensor_tensor(
                        t2[64:128],
                        ps[0:64],
                        bqkv_sb[0:64, ch : ch + 1],
                        sin_sb[0:64, tab, ts_],
                        ALU.add,
                        ALU.mult,
                    )
                    nc.vector.tensor_add(dest, m1, t2)

        # ---- phase 1b: v projection (transposed) + PE transpose to [tok, hs] ----
        with (
            tc.tile_pool(name="pv", bufs=4, space="PSUM") as pv,
            tc.tile_pool(name="ptr", bufs=3, space="PSUM") as ptr,
        ):
            vps = [
                [pv.tile([128, 512], F32, tag="vp", name="vp") for _ in range(NSL)]
                for _ in range(HL)
            ]
            for kc in range(KC):
                wt = wtile.tile([128, 256], F16, tag="wv")
                nc.gpsimd.dma_start(out=wt, in_=wqkv[kc, :, 512:768])
                for h in range(HL):
                    for sl in range(NSL):
                        nc.tensor.matmul(
                            vps[h][sl],
                            wt[:, h * 128 : (h + 1) * 128],
                            xt[kc][:, sl * 512 : (sl + 1) * 512],
                            start=(kc == 0),
                            stop=(kc == KC - 1),
                        )
            for h in range(HL):
                for sl in range(NSL):
                    nc.scalar.activation(
                        out=vT_sb[:, h, sl * 512 : (sl + 1) * 512],
                        in_=vps[h][sl],
                        func=AF.Identity,
                        bias=bqkv_sb[:, 4 + h : 5 + h],
                    )
                nc.sync.dma_start(out=v_newT[h, :, :], in_=vT_sb[:, h, :])
                for tc8 in range(NT):
                    tp = ptr.tile([128, 128], F32, tag="vtp")
                    nc.tensor.transpose(
                        tp, vT_sb[:, h, tc8 * 128 : (tc8 + 1) * 128], ident_sb
                    )
                    nc.scalar.copy(
                        out=v_sb[:, h, OFF // 128 + tc8, :], in_=tp
                    )

        # ---- phase 2: attention, scores^T layout [u, t] ----
        PIPE = 3
        with (
            tc.tile_pool(name="psc", bufs=4, space="PSUM") as psc,
            tc.tile_pool(name="pyt", bufs=2, space="PSUM") as pyt,
            tc.tile_pool(name="pss", bufs=2, space="PSUM") as pss,
        ):
            for h in range(HL):
                for sl in range(NSL):
                    nu = _nu_of_slab(sl)
                    ts_ = slice(sl * 512, (sl + 1) * 512)
                    ytp = pyt.tile([128, 512], F32, tag="ytp")
                    ssp = pss.tile([128, 512], F32, tag="ssp")
                    pts = [None] * nu

                    def consume(uc):
                        nc.tensor.matmul(
                            ytp,
                            v_sb[:, h, uc, :],
                            pts[uc],
                            start=(uc == 0),
                            stop=(uc == nu - 1),
                        )
                        nc.tensor.matmul(
                            ssp,
                            ones_sb,
                            pts[uc],
                            start=(uc == 0),
                            stop=(uc == nu - 1),
                        )

                    for uc in range(nu):
                        scp = psc.tile([128, 512], F32, tag="scp")
                        nc.tensor.matmul(
                            scp,
                            kT_sb[:, h, uc * 128 : (uc + 1) * 128],
                            qT_sb[:, h, ts_],
                            start=True,
                            stop=True,
                        )
                        off = sl * 512 + 1408 - uc * 128
                        sbt = sbtp.tile([128, 512], F32, tag="sbt")
                        nc.vector.tensor_add(
                            sbt, scp, wband[:, h, off : off + 512]
                        )
                        pt = ppool.tile([128, 512], BF16, tag="pt")
                        nc.scalar.activation(out=pt, in_=sbt, func=AF.Exp)
                        pts[uc] = pt
                        if uc >= PIPE:
                            consume(uc - PIPE)
                    for uc in range(max(0, nu - PIPE), nu):
                        consume(uc)

                    inv = tmp.tile([128, 512], F32, tag="inv")
                    nc.vector.reciprocal(out=inv, in_=ssp)
                    nc.vector.tensor_mul(yT_sb[:, h, ts_], ytp, inv)

        # ---- phase 3: output projection (partial y) ----
        with tc.tile_pool(name="po", bufs=4, space="PSUM") as po:
            for tc8 in range(NT):
                for ns in range(4):
                    pop = po.tile([128, 512], F32, tag="pop")
                    for h in range(HL):
                        nc.tensor.matmul(
                            pop,
                            yT_sb[:, h, tc8 * 128 : (tc8 + 1) * 128],
                            wp_sb[:, h, ns * 512 : (ns + 1) * 512],
                            start=(h == 0),
                            stop=(h == HL - 1),
                        )
                    ystg = tmp.tile([128, 512], F32, tag="ystg")
                    nc.scalar.copy(out=ystg, in_=pop)
                    nc.sync.dma_start(
                        out=y_out[
                            tc8 * 128 : (tc8 + 1) * 128, ns * 512 : (ns + 1) * 512
                        ],
                        in_=ystg,
                    )

        # ---- phase 4: new-k output ----
        for h in range(HL):
            nc.sync.dma_start(out=kT_new[h, :, :], in_=kT_sb[:, h, OFF:TF])

    nc.compile()
    return nc


def kernel(x, cached_k, cached_v, W_attn, b_attn, W_proj, b_proj, decay_raw):
    x = np.asarray(x, np.float32)
    cached_k = np.asarray(cached_k, np.float32)
    cached_v = np.asarray(cached_v, np.float32)
    W_attn = np.asarray(W_attn, np.float32)
    b_attn = np.asarray(b_attn, np.float32)
    W_proj = np.asarray(W_proj, np.float32)
    b_proj = np.asarray(b_proj, np.float32)
    decay_raw = np.asarray(decay_raw, np.float32)

    scale = np.float32(1.0 / np.sqrt(HS))
    xT = np.ascontiguousarray(x[0].T).reshape(KC, 128, T).astype(np.float16)

    # RoPE tables (match reference fp32 computation); q tables carry the
    # 1/sqrt(hs) score scale.
    pos = np.arange(OFF, OFF + T).astype(np.float32)
    inv_freq = (
        np.float32(1.0)
        / (np.float32(10000.0) ** (np.arange(HS // 2, dtype=np.float32) / np.float32(HS // 2)))
    ).astype(np.float32)
    ang = (pos[:, None] * inv_freq[None, :]).astype(np.float32)  # [T, 64]
    cos = np.cos(ang).astype(np.float32).T  # [64, T]
    sin = np.sin(ang).astype(np.float32).T
    cos_full = np.concatenate([cos, cos], axis=0)       # [128, T]
    sin_full = np.concatenate([sin, -sin], axis=0)      # [128, T]
    cosf = np.ascontiguousarray(
        np.stack([cos_full * scale, cos_full], axis=1)
    )  # [128, 2, T]
    sinf = np.ascontiguousarray(np.stack([sin_full * scale, sin_full], axis=1))

    decay = np.log1p(np.exp(decay_raw.astype(np.float64))).astype(np.float32)

    ii = np.arange(BW)[None, :] - np.arange(128)[:, None] + CMIN  # d = i - p - 384
    dpos = np.maximum(ii, 0).astype(np.float64)
    logd = np.log1p(dpos)  # [128, BW]

    nc = _build_nc()

    in_maps = []
    for c in range(NCORES):
        g0 = HL * c
        qcols = W_attn[:, g0 * HS : (g0 + HL) * HS]  # scale lives in q rope tables
        kcols = W_attn[:, C + g0 * HS : C + (g0 + HL) * HS]
        vcols = W_attn[:, 2 * C + g0 * HS : 2 * C + (g0 + HL) * HS]
        wqkv_c = (
            np.ascontiguousarray(np.concatenate([qcols, kcols, vcols], axis=1))
            .reshape(KC, 128, 6 * 128)
            .astype(np.float16)
        )

        bq = b_attn[g0 * HS : (g0 + HL) * HS]
        bk = b_attn[C + g0 * HS : C + (g0 + HL) * HS]
        bv = b_attn[2 * C + g0 * HS : 2 * C + (g0 + HL) * HS]
        bqkv_c = np.concatenate([bq, bk, bv]).reshape(6, 128).T.copy()  # [128, 6]

        wp_c = (
            np.ascontiguousarray(W_proj[g0 * HS : (g0 + HL) * HS, :])
            .reshape(HL, 128, C)
            .astype(ml_dtypes.bfloat16)
        )
        ckT_c = np.ascontiguousarray(cached_k[0, g0 : g0 + HL].transpose(0, 2, 1))
        cv_c = np.ascontiguousarray(
            cached_v[0, g0 : g0 + HL].reshape(HL, OFF // 128, 128, HS)
        ).astype(ml_dtypes.bfloat16)

        wband_c = np.empty((HL, 128, BW), np.float32)
        for l in range(HL):
            val = -np.log1p(np.float64(decay[g0 + l]) * logd)
            wband_c[l] = np.where(ii >= 0, val, MASKVAL).astype(np.float32)

        in_maps.append(
            {
                "xT": xT,
                "wqkv": wqkv_c,
                "bqkv": bqkv_c,
                "cosf": cosf,
                "sinf": sinf,
                "wp": wp_c,
                "ckT": ckT_c,
                "cv": cv_c,
                "wband_in": wband_c,
            }
        )

    res = run_bass_kernel_spmd(nc, in_maps, core_ids=list(range(NCORES)))
    results = res.results
    kernel._last = results

    # ---- gather ----
    y = np.zeros((T, C), np.float64)
    for c in range(NCORES):
        y += results[c]["y_out"].astype(np.float64)
    y = (y.astype(np.float32) + b_proj[None, :]).reshape(1, T, C)

    k_full = np.empty((1, H, TF, HS), np.float32)
    v_full = np.empty((1, H, TF, HS), np.float32)
    k_full[0, :, :OFF] = cached_k[0]
    v_full[0, :, :OFF] = cached_v[0]
    for c in range(NCORES):
        kT_n = results[c]["kT_new"]   # [HL, 128, T]
        vT_n = results[c]["v_newT"]   # [HL, 128, T]
        for l in range(HL):
            g = HL * c + l
            k_full[0, g, OFF:] = kT_n[l].T
            v_full[0, g, OFF:] = vT_n[l].T

    return (y, k_full, v_full)


# revision 17
# speedup vs baseline: 1.2114x; 1.1083x over previous
"""Causal self-attention with KV cache + per-head distance-decay bias, on 8 trn2 cores.

Sharding: tensor-parallel over heads (16 heads -> 2 per core). Each core:
  - projects full x with its shard of W_attn (fp16 matmuls, all outputs transposed
    [feature, token]), applies RoPE to q/k (scale 1/sqrt(hs) folded into q's
    cos/sin tables), PE-transposes v back to [token, feature] in bf16
  - computes scores^T [keys, queries] tiles in fp32r, adds a host-precomputed
    Toeplitz bias band (causal mask + -log1p(decay*log1p(dist))), exp -> bf16,
    attention @ V in bf16 (denominators via an all-ones matmul)
  - projects with its shard of W_proj rows (bf16) -> partial y
Host sums the 8 partial y outputs and assembles the full k/v caches.
"""

import numpy as np
import ml_dtypes
from contextlib import ExitStack

import concourse.bass as bass
import concourse.mybir as mybir
import concourse.tile as tile
from concourse import bacc
from concourse.masks import make_identity
from concourse.bass_utils import run_bass_kernel_spmd

F32 = mybir.dt.float32
F32R = mybir.dt.float32r
F16 = mybir.dt.float16
BF16 = mybir.dt.bfloat16
AF = mybir.ActivationFunctionType
ALU = mybir.AluOpType

H, T, OFF, C, HS = 16, 1024, 1024, 2048, 128
NCORES, HL = 8, 2          # heads per core
TF = OFF + T               # 2048 full key length
KC = C // 128              # 16 contraction chunks
NT = T // 128              # 8 t-chunks
NSL = T // 512             # 2 t-slabs
NU = TF // 128             # 16 u-chunks
BW = 2432                  # bias band width: (c_max - c_min) + 512
CMIN = -384
MASKVAL = -10000.0


def _nu_of_slab(sl):
    # u-chunks uc with c = sl*512 + 1024 - uc*128 >= -384
    return 12 if sl == 0 else 16


def _build_nc():
    nc = bacc.Bacc(
        "TRN2", target_bir_lowering=False, debug=False, enable_asserts=False
    )

    xT = nc.dram_tensor("xT", [KC, 128, T], F16, kind="ExternalInput")
    wqkv = nc.dram_tensor("wqkv", [KC, 128, 6 * 128], F16, kind="ExternalInput")
    bqkv = nc.dram_tensor("bqkv", [128, 6], F32, kind="ExternalInput")
    cosf = nc.dram_tensor("cosf", [128, 2, T], F32, kind="ExternalInput")
    sinf = nc.dram_tensor("sinf", [128, 2, T], F32, kind="ExternalInput")
    wp = nc.dram_tensor("wp", [HL, 128, C], BF16, kind="ExternalInput")
    ckT = nc.dram_tensor("ckT", [HL, 128, OFF], F32R, kind="ExternalInput")
    cv = nc.dram_tensor("cv", [HL, OFF // 128, 128, HS], BF16, kind="ExternalInput")
    wband_in = nc.dram_tensor("wband_in", [HL, 128, BW], F32, kind="ExternalInput")

    y_out = nc.dram_tensor("y_out", [T, C], F32, kind="ExternalOutput")
    kT_new = nc.dram_tensor("kT_new", [HL, 128, T], F32R, kind="ExternalOutput")
    v_newT = nc.dram_tensor("v_newT", [HL, 128, T], F32, kind="ExternalOutput")

    with tile.TileContext(nc) as tc, ExitStack() as ctx:
        const = ctx.enter_context(tc.tile_pool(name="const", bufs=1))
        state = ctx.enter_context(tc.tile_pool(name="state", bufs=1))
        xt_pool = ctx.enter_context(tc.tile_pool(name="xt", bufs=KC))
        wtile = ctx.enter_context(tc.tile_pool(name="wtile", bufs=3))
        tmp = ctx.enter_context(tc.tile_pool(name="tmp", bufs=2))
        sbtp = ctx.enter_context(tc.tile_pool(name="sbtp", bufs=4))
        ppool = ctx.enter_context(tc.tile_pool(name="ppool", bufs=6))

        # ---- constants ----
        cos_sb = const.tile([128, 2, T], F32)   # [:, 0, :] q-scaled, [:, 1, :] k
        sin_sb = const.tile([128, 2, T], F32)
        bqkv_sb = const.tile([128, 6], F32)
        ones_sb = const.tile([128, 128], BF16)
        ident_sb = const.tile([128, 128], F32)
        wp_sb = const.tile([128, HL, C], BF16)
        nc.sync.dma_start(out=cos_sb, in_=cosf[:, :, :])
        nc.sync.dma_start(out=sin_sb, in_=sinf[:, :, :])
        nc.sync.dma_start(out=bqkv_sb, in_=bqkv[:, :])
        nc.vector.memset(ones_sb, 1.0)
        make_identity(nc, ident_sb)
        nc.sync.dma_start(
            out=wp_sb, in_=wp[:, :, :].rearrange("h p c -> p h c")
        )

        # ---- persistent activations ----
        kT_sb = state.tile([128, HL, TF], F32R)     # [hs, head, keys]
        qT_sb = state.tile([128, HL, T], F32R)      # [hs, head, queries]
        vT_sb = state.tile([128, HL, T], F32)       # [hs, head, new tokens]
        v_sb = state.tile([128, HL, NU, HS], BF16)  # [tok%128, head, uc, hs]
        yT_sb = state.tile([128, HL, T], BF16)      # [hs, head, queries]
        wband = state.tile([128, HL, BW], F32)      # Toeplitz bias band per head

        for h in range(HL):
            nc.sync.dma_start(out=kT_sb[:, h, 0:OFF], in_=ckT[h, :, :])
            nc.sync.dma_start(
                out=v_sb[:, h, 0 : OFF // 128, :],
                in_=cv[h, :, :, :].rearrange("u p d -> p u d"),
            )
        nc.sync.dma_start(
            out=wband, in_=wband_in[:, :, :].rearrange("h p i -> p h i")
        )

        # ---- load xT resident ----
        xt = []
        for kc in range(KC):
            t_ = xt_pool.tile([128, T], F16, tag="xt")
            nc.sync.dma_start(out=t_, in_=xT[kc, :, :])
            xt.append(t_)

        # ---- phase 1a: q/k projection (transposed out) + RoPE ----
        # chunk order in wqkv: 0=q_h0, 1=q_h1, 2=k_h0, 3=k_h1, 4=v_h0, 5=v_h1
        with (
            tc.tile_pool(name="pmm", bufs=8, space="PSUM") as pmm,
        ):
            qkp = [
                [pmm.tile([128, 512], F32, tag="mmps", name="qkp") for _ in range(NSL)]
                for _ in range(4)
            ]
            for kc in range(KC):
                wt = wtile.tile([128, 512], F16, tag="wqk")
                nc.gpsimd.dma_start(out=wt, in_=wqkv[kc, :, 0:512])
                for ch in range(4):
                    for sl in range(NSL):
                        nc.tensor.matmul(
                            qkp[ch][sl],
                            wt[:, ch * 128 : (ch + 1) * 128],
                            xt[kc][:, sl * 512 : (sl + 1) * 512],
                            start=(kc == 0),
                            stop=(kc == KC - 1),
                        )
            for ch in range(4):
                h = ch % 2
                is_q = ch < 2
                tab = 0 if is_q else 1  # q tables carry the 1/sqrt(hs) scale
                for sl in range(NSL):
                    ts_ = slice(sl * 512, (sl + 1) * 512)
                    if is_q:
                        dest = qT_sb[:, h, ts_]
                    else:
                        dest = kT_sb[:, h, OFF + sl * 512 : OFF + (sl + 1) * 512]
                    ps = qkp[ch][sl]
                    bcol = bqkv_sb[:, ch : ch + 1]
                    m1 = tmp.tile([128, 512], F32, tag="ropem1")
                    t2 = tmp.tile([128, 512], F32, tag="ropet2")
                    nc.vector.scalar_tensor_tensor(
                        m1, ps, bcol, cos_sb[:, tab, ts_], ALU.add, ALU.mult
                    )
                    nc.vector.scalar_tensor_tensor(
                        t2[0:64],
                        ps[64:128],
                        bqkv_sb[64:128, ch : ch + 1],
                        sin_sb[64:128, tab, ts_],
                        ALU.add,
                        ALU.mult,
                    )
                    nc.vector.scalar_tensor_tensor(
                        t2[64:128],
                        ps[0:64],
                        bqkv_sb[0:64, ch : ch + 1],
                        sin_sb[0:64, tab, ts_],
                        ALU.add,
                        ALU.mult,
                    )
                    nc.vector.tensor_add(dest, m1, t2)

            # ---- phase 1b: v projection (transposed), reusing freed qk slots ----
            vps = [
                [pmm.tile([128, 512], F32, tag="mmps", name="vp") for _ in range(NSL)]
                for _ in range(HL)
            ]
            for kc in range(KC):
                wt = wtile.tile([128, 256], F16, tag="wv")
                nc.gpsimd.dma_start(out=wt, in_=wqkv[kc, :, 512:768])
                for h in range(HL):
                    for sl in range(NSL):
                        nc.tensor.matmul(
                            vps[h][sl],
                            wt[:, h * 128 : (h + 1) * 128],
                            xt[kc][:, sl * 512 : (sl + 1) * 512],
                            start=(kc == 0),
                            stop=(kc == KC - 1),
                        )
            for h in range(HL):
                for sl in range(NSL):
                    nc.scalar.activation(
                        out=vT_sb[:, h, sl * 512 : (sl + 1) * 512],
                        in_=vps[h][sl],
                        func=AF.Identity,
                        bias=bqkv_sb[:, 4 + h : 5 + h],
                    )
                nc.sync.dma_start(out=v_newT[h, :, :], in_=vT_sb[:, h, :])

        with tc.tile_pool(name="ptr", bufs=2, space="PSUM") as ptr:
            for h in range(HL):
                for tc8 in range(NT):
                    tp = ptr.tile([128, 128], F32, tag="vtp")
                    nc.tensor.transpose(
                        tp, vT_sb[:, h, tc8 * 128 : (tc8 + 1) * 128], ident_sb
                    )
                    nc.scalar.copy(
                        out=v_sb[:, h, OFF // 128 + tc8, :], in_=tp
                    )

        # ---- phase 2: attention, scores^T layout [u, t], both t-slabs per uc ----
        PIPE = 2
        with (
            tc.tile_pool(name="psc", bufs=4, space="PSUM") as psc,
            tc.tile_pool(name="pyt", bufs=2, space="PSUM") as pyt,
            tc.tile_pool(name="pss", bufs=2, space="PSUM") as pss,
        ):
            NU0, NU1 = _nu_of_slab(0), _nu_of_slab(1)
            for h in range(HL):
                ytps = [pyt.tile([128, 512], F32, tag="ytp", name="ytp") for _ in range(NSL)]
                ssps = [pss.tile([128, 512], F32, tag="ssp", name="ssp") for _ in range(NSL)]
                pts = {}

                def slabs_of(uc):
                    return [0, 1] if uc < NU0 else [1]

                def consume(uc):
                    for sl in slabs_of(uc):
                        nu = NU0 if sl == 0 else NU1
                        pt, col = pts[uc]
                        rhs = pt[:, col[sl][0] : col[sl][1]]
                        nc.tensor.matmul(
                            ytps[sl], v_sb[:, h, uc, :], rhs,
                            start=(uc == 0), stop=(uc == nu - 1),
                        )
                        nc.tensor.matmul(
                            ssps[sl], ones_sb, rhs,
                            start=(uc == 0), stop=(uc == nu - 1),
                        )

                for uc in range(NU1):
                    sls = slabs_of(uc)
                    if len(sls) == 2:
                        sbt = sbtp.tile([128, 2, 512], F32, tag="sbt2")
                        pt = ppool.tile([128, 2, 512], BF16, tag="pt2")
                        cols = {0: (0, 512), 1: (512, 1024)}
                    else:
                        sbt = sbtp.tile([128, 1, 512], F32, tag="sbt1")
                        pt = ppool.tile([128, 1, 512], BF16, tag="pt1")
                        cols = {1: (0, 512)}
                    for j, sl in enumerate(sls):
                        scp = psc.tile([128, 512], F32, tag="scp")
                        nc.tensor.matmul(
                            scp,
                            kT_sb[:, h, uc * 128 : (uc + 1) * 128],
                            qT_sb[:, h, sl * 512 : (sl + 1) * 512],
                            start=True, stop=True,
                        )
                        off = sl * 512 + 1408 - uc * 128
                        nc.vector.tensor_add(
                            sbt[:, j, :], scp, wband[:, h, off : off + 512]
                        )
                    nc.scalar.activation(
                        out=pt.rearrange("p a b -> p (a b)"),
                        in_=sbt.rearrange("p a b -> p (a b)"),
                        func=AF.Exp,
                    )
                    pts[uc] = (pt.rearrange("p a b -> p (a b)"), cols)
                    if uc >= PIPE:
                        consume(uc - PIPE)
                for uc in range(max(0, NU1 - PIPE), NU1):
                    consume(uc)

                for sl in range(NSL):
                    ts_ = slice(sl * 512, (sl + 1) * 512)
                    inv = tmp.tile([128, 512], F32, tag="inv")
                    nc.vector.reciprocal(out=inv, in_=ssps[sl])
                    nc.vector.tensor_mul(yT_sb[:, h, ts_], ytps[sl], inv)

        # ---- phase 3: output projection (partial y) ----
        with tc.tile_pool(name="po", bufs=4, space="PSUM") as po:
            for tc8 in range(NT):
                for ns in range(4):
                    pop = po.tile([128, 512], F32, tag="pop")
                    for h in range(HL):
                        nc.tensor.matmul(
                            pop,
                            yT_sb[:, h, tc8 * 128 : (tc8 + 1) * 128],
                            wp_sb[:, h, ns * 512 : (ns + 1) * 512],
                            start=(h == 0),
                            stop=(h == HL - 1),
                        )
                    ystg = tmp.tile([128, 512], F32, tag="ystg")
                    nc.scalar.copy(out=ystg, in_=pop)
                    nc.sync.dma_start(
                        out=y_out[
                            tc8 * 128 : (tc8 + 1) * 128, ns * 512 : (ns + 1) * 512
                        ],
                        in_=ystg,
                    )

        # ---- phase 4: new-k output ----
        for h in range(HL):
            nc.sync.dma_start(out=kT_new[h, :, :], in_=kT_sb[:, h, OFF:TF])

    nc.compile()
    return nc


def kernel(x, cached_k, cached_v, W_attn, b_attn, W_proj, b_proj, decay_raw):
    x = np.asarray(x, np.float32)
    cached_k = np.asarray(cached_k, np.float32)
    cached_v = np.asarray(cached_v, np.float32)
    W_attn = np.asarray(W_attn, np.float32)
    b_attn = np.asarray(b_attn, np.float32)
    W_proj = np.asarray(W_proj, np.float32)
    b_proj = np.asarray(b_proj, np.float32)
    decay_raw = np.asarray(decay_raw, np.float32)

    scale = np.float32(1.0 / np.sqrt(HS))
    xT = np.ascontiguousarray(x[0].T).reshape(KC, 128, T).astype(np.float16)

    # RoPE tables (match reference fp32 computation); q tables carry the
    # 1/sqrt(hs) score scale.
    pos = np.arange(OFF, OFF + T).astype(np.float32)
    inv_freq = (
        np.float32(1.0)
        / (np.float32(10000.0) ** (np.arange(HS // 2, dtype=np.float32) / np.float32(HS // 2)))
    ).astype(np.float32)
    ang = (pos[:, None] * inv_freq[None, :]).astype(np.float32)  # [T, 64]
    cos = np.cos(ang).astype(np.float32).T  # [64, T]
    sin = np.sin(ang).astype(np.float32).T
    cos_full = np.concatenate([cos, cos], axis=0)       # [128, T]
    sin_full = np.concatenate([sin, -sin], axis=0)      # [128, T]
    cosf = np.ascontiguousarray(
        np.stack([cos_full * scale, cos_full], axis=1)
    )  # [128, 2, T]
    sinf = np.ascontiguousarray(np.stack([sin_full * scale, sin_full], axis=1))

    decay = np.log1p(np.exp(decay_raw.astype(np.float64))).astype(np.float32)

    ii = np.arange(BW)[None, :] - np.arange(128)[:, None] + CMIN  # d = i - p - 384
    dpos = np.maximum(ii, 0).astype(np.float64)
    logd = np.log1p(dpos)  # [128, BW]

    nc = _build_nc()

    in_maps = []
    for c in range(NCORES):
        g0 = HL * c
        qcols = W_attn[:, g0 * HS : (g0 + HL) * HS]  # scale lives in q rope tables
        kcols = W_attn[:, C + g0 * HS : C + (g0 + HL) * HS]
        vcols = W_attn[:, 2 * C + g0 * HS : 2 * C + (g0 + HL) * HS]
        wqkv_c = (
            np.ascontiguousarray(np.concatenate([qcols, kcols, vcols], axis=1))
            .reshape(KC, 128, 6 * 128)
            .astype(np.float16)
        )

        bq = b_attn[g0 * HS : (g0 + HL) * HS]
        bk = b_attn[C + g0 * HS : C + (g0 + HL) * HS]
        bv = b_attn[2 * C + g0 * HS : 2 * C + (g0 + HL) * HS]
        bqkv_c = np.concatenate([bq, bk, bv]).reshape(6, 128).T.copy()  # [128, 6]

        wp_c = (
            np.ascontiguousarray(W_proj[g0 * HS : (g0 + HL) * HS, :])
            .reshape(HL, 128, C)
            .astype(ml_dtypes.bfloat16)
        )
        ckT_c = np.ascontiguousarray(cached_k[0, g0 : g0 + HL].transpose(0, 2, 1))
        cv_c = np.ascontiguousarray(
            cached_v[0, g0 : g0 + HL].reshape(HL, OFF // 128, 128, HS)
        ).astype(ml_dtypes.bfloat16)

        wband_c = np.empty((HL, 128, BW), np.float32)
        for l in range(HL):
            val = -np.log1p(np.float64(decay[g0 + l]) * logd)
            wband_c[l] = np.where(ii >= 0, val, MASKVAL).astype(np.float32)

        in_maps.append(
            {
                "xT": xT,
                "wqkv": wqkv_c,
                "bqkv": bqkv_c,
                "cosf": cosf,
                "sinf": sinf,
                "wp": wp_c,
                "ckT": ckT_c,
                "cv": cv_c,
                "wband_in": wband_c,
            }
        )

    res = run_bass_kernel_spmd(nc, in_maps, core_ids=list(range(NCORES)))
    results = res.results
    kernel._last = results

    # ---- gather ----
    y = np.zeros((T, C), np.float64)
    for c in range(NCORES):
        y += results[c]["y_out"].astype(np.float64)
    y = (y.astype(np.float32) + b_proj[None, :]).reshape(1, T, C)

    k_full = np.empty((1, H, TF, HS), np.float32)
    v_full = np.empty((1, H, TF, HS), np.float32)
    k_full[0, :, :OFF] = cached_k[0]
    v_full[0, :, :OFF] = cached_v[0]
    for c in range(NCORES):
        kT_n = results[c]["kT_new"]   # [HL, 128, T]
        vT_n = results[c]["v_newT"]   # [HL, 128, T]
        for l in range(HL):
            g = HL * c + l
            k_full[0, g, OFF:] = kT_n[l].T
            v_full[0, g, OFF:] = vT_n[l].T

    return (y, k_full, v_full)


# revision 18
# speedup vs baseline: 1.4814x; 1.2230x over previous
"""Causal self-attention with KV cache + per-head distance-decay bias, on 8 trn2 cores.

Sharding: tensor-parallel over heads (16 heads -> 2 per core). Each core:
  - projects full x with its shard of W_attn (fp16 matmuls, all outputs transposed
    [feature, token]), applies RoPE to q/k (scale 1/sqrt(hs) folded into q's
    cos/sin tables), PE-transposes v back to [token, feature] in bf16
  - computes scores^T [keys, queries] tiles in fp32r, adds a host-precomputed
    Toeplitz bias band (causal mask + -log1p(decay*log1p(dist))), exp -> bf16,
    attention @ V in bf16 (denominators via an all-ones matmul)
  - projects with its shard of W_proj rows (bf16) -> partial y
Host sums the 8 partial y outputs and assembles the full k/v caches.
"""

import numpy as np
import ml_dtypes
from contextlib import ExitStack

import concourse.bass as bass
import concourse.mybir as mybir
import concourse.tile as tile
from concourse import bacc
from concourse.masks import make_identity
from concourse.bass_utils import run_bass_kernel_spmd

F32 = mybir.dt.float32
F32R = mybir.dt.float32r
F16 = mybir.dt.float16
BF16 = mybir.dt.bfloat16
AF = mybir.ActivationFunctionType
ALU = mybir.AluOpType

H, T, OFF, C, HS = 16, 1024, 1024, 2048, 128
NCORES, HL = 8, 2          # heads per core
TF = OFF + T               # 2048 full key length
KC = C // 128              # 16 contraction chunks
NT = T // 128              # 8 t-chunks
NSL = T // 512             # 2 t-slabs
NU = TF // 128             # 16 u-chunks
BW = 2432                  # bias band width: (c_max - c_min) + 512
CMIN = -384
MASKVAL = -10000.0


def _nu_of_slab(sl):
    # u-chunks uc with c = sl*512 + 1024 - uc*128 >= -384
    return 12 if sl == 0 else 16


def _build_nc():
    nc = bacc.Bacc(
        "TRN2", target_bir_lowering=False, debug=False, enable_asserts=False
    )

    xT = nc.dram_tensor("xT", [KC, 128, T], F16, kind="ExternalInput")
    wqkv = nc.dram_tensor("wqkv", [KC, 128, 6 * 128], F16, kind="ExternalInput")
    bqkv = nc.dram_tensor("bqkv", [128, 6], F32, kind="ExternalInput")
    cosf = nc.dram_tensor("cosf", [128, 2, T], F32, kind="ExternalInput")
    sinf = nc.dram_tensor("sinf", [128, 2, T], F32, kind="ExternalInput")
    wp = nc.dram_tensor("wp", [HL, 128, C], BF16, kind="ExternalInput")
    ckT = nc.dram_tensor("ckT", [HL, 128, OFF], F16, kind="ExternalInput")
    cv = nc.dram_tensor("cv", [HL, OFF // 128, 128, HS], BF16, kind="ExternalInput")
    wband_in = nc.dram_tensor("wband_in", [HL, 128, BW], F32, kind="ExternalInput")

    y_out = nc.dram_tensor("y_out", [T, C], BF16, kind="ExternalOutput")
    kT_new = nc.dram_tensor("kT_new", [HL, 128, T], F16, kind="ExternalOutput")
    v_newT = nc.dram_tensor("v_newT", [HL, 128, T], F32, kind="ExternalOutput")

    with tile.TileContext(nc) as tc, ExitStack() as ctx:
        const = ctx.enter_context(tc.tile_pool(name="const", bufs=1))
        state = ctx.enter_context(tc.tile_pool(name="state", bufs=1))
        xt_pool = ctx.enter_context(tc.tile_pool(name="xt", bufs=KC))
        wtile = ctx.enter_context(tc.tile_pool(name="wtile", bufs=3))
        tmp = ctx.enter_context(tc.tile_pool(name="tmp", bufs=2))
        sbtp = ctx.enter_context(tc.tile_pool(name="sbtp", bufs=4))
        ystgp = ctx.enter_context(tc.tile_pool(name="ystgp", bufs=4))
        ppool = ctx.enter_context(tc.tile_pool(name="ppool", bufs=6))

        # ---- constants ----
        cos_sb = const.tile([128, 2, T], F32)   # [:, 0, :] q-scaled, [:, 1, :] k
        sin_sb = const.tile([128, 2, T], F32)
        bqkv_sb = const.tile([128, 6], F32)
        ones_sb = const.tile([128, 128], BF16)
        ident_sb = const.tile([128, 128], F32)
        wp_sb = const.tile([128, HL, C], BF16)
        # ---- persistent activations ----
        kT_sb = state.tile([128, HL, TF], F16)     # [hs, head, keys]
        qT_sb = state.tile([128, HL, T], F16)      # [hs, head, queries]
        vT_sb = state.tile([128, HL, T], F32)       # [hs, head, new tokens]
        v_sb = state.tile([128, HL, NU, HS], BF16)  # [tok%128, head, uc, hs]
        yT_sb = state.tile([128, HL, T], BF16)      # [hs, head, queries]
        wband = state.tile([128, HL, BW], F32)      # Toeplitz bias band per head


        # ---- load xT resident (first in DMA queue order) ----
        xt = []
        for kc in range(KC):
            t_ = xt_pool.tile([128, T], F16, tag="xt")
            nc.sync.dma_start(out=t_, in_=xT[kc, :, :])
            xt.append(t_)

        # const / cached loads (issued after xt so the first matmuls aren't
        # queued behind ~13 MB of DMA)
        nc.sync.dma_start(out=cos_sb, in_=cosf[:, :, :])
        nc.sync.dma_start(out=sin_sb, in_=sinf[:, :, :])
        nc.sync.dma_start(out=bqkv_sb, in_=bqkv[:, :])
        nc.vector.memset(ones_sb, 1.0)
        make_identity(nc, ident_sb)
        for h in range(HL):
            nc.sync.dma_start(out=kT_sb[:, h, 0:OFF], in_=ckT[h, :, :])
            nc.sync.dma_start(
                out=v_sb[:, h, 0 : OFF // 128, :],
                in_=cv[h, :, :, :].rearrange("u p d -> p u d"),
            )
        nc.sync.dma_start(
            out=wband, in_=wband_in[:, :, :].rearrange("h p i -> p h i")
        )
        nc.sync.dma_start(
            out=wp_sb, in_=wp[:, :, :].rearrange("h p c -> p h c")
        )

        # ---- phase 1a: q/k projection (transposed out) + RoPE ----
        # chunk order in wqkv: 0=q_h0, 1=q_h1, 2=k_h0, 3=k_h1, 4=v_h0, 5=v_h1
        with (
            tc.tile_pool(name="pmm", bufs=8, space="PSUM") as pmm,
        ):
            qkp = [
                [pmm.tile([128, 512], F32, tag="mmps", name="qkp") for _ in range(NSL)]
                for _ in range(4)
            ]
            for kc in range(KC):
                wt = wtile.tile([128, 512], F16, tag="wqk")
                nc.gpsimd.dma_start(out=wt, in_=wqkv[kc, :, 0:512])
                for ch in range(4):
                    for sl in range(NSL):
                        nc.tensor.matmul(
                            qkp[ch][sl],
                            wt[:, ch * 128 : (ch + 1) * 128],
                            xt[kc][:, sl * 512 : (sl + 1) * 512],
                            start=(kc == 0),
                            stop=(kc == KC - 1),
                        )
            for ch in range(4):
                h = ch % 2
                is_q = ch < 2
                tab = 0 if is_q else 1  # q tables carry the 1/sqrt(hs) scale
                for sl in range(NSL):
                    ts_ = slice(sl * 512, (sl + 1) * 512)
                    if is_q:
                        dest = qT_sb[:, h, ts_]
                    else:
                        dest = kT_sb[:, h, OFF + sl * 512 : OFF + (sl + 1) * 512]
                    ps = qkp[ch][sl]
                    bcol = bqkv_sb[:, ch : ch + 1]
                    m1 = tmp.tile([128, 512], F32, tag="ropem1")
                    t2 = tmp.tile([128, 512], F32, tag="ropet2")
                    nc.vector.scalar_tensor_tensor(
                        m1, ps, bcol, cos_sb[:, tab, ts_], ALU.add, ALU.mult
                    )
                    nc.vector.scalar_tensor_tensor(
                        t2[0:64],
                        ps[64:128],
                        bqkv_sb[64:128, ch : ch + 1],
                        sin_sb[64:128, tab, ts_],
                        ALU.add,
                        ALU.mult,
                    )
                    nc.vector.scalar_tensor_tensor(
                        t2[64:128],
                        ps[0:64],
                        bqkv_sb[0:64, ch : ch + 1],
                        sin_sb[0:64, tab, ts_],
                        ALU.add,
                        ALU.mult,
                    )
                    nc.vector.tensor_add(dest, m1, t2)

            # ---- phase 1b: v projection (transposed), reusing freed qk slots ----
            vps = [
                [pmm.tile([128, 512], F32, tag="mmps", name="vp") for _ in range(NSL)]
                for _ in range(HL)
            ]
            for kc in range(KC):
                wt = wtile.tile([128, 256], F16, tag="wv")
                nc.gpsimd.dma_start(out=wt, in_=wqkv[kc, :, 512:768])
                for h in range(HL):
                    for sl in range(NSL):
                        nc.tensor.matmul(
                            vps[h][sl],
                            wt[:, h * 128 : (h + 1) * 128],
                            xt[kc][:, sl * 512 : (sl + 1) * 512],
                            start=(kc == 0),
                            stop=(kc == KC - 1),
                        )
            for h in range(HL):
                for sl in range(NSL):
                    nc.scalar.activation(
                        out=vT_sb[:, h, sl * 512 : (sl + 1) * 512],
                        in_=vps[h][sl],
                        func=AF.Identity,
                        bias=bqkv_sb[:, 4 + h : 5 + h],
                    )
                nc.sync.dma_start(out=v_newT[h, :, :], in_=vT_sb[:, h, :])

        with tc.tile_pool(name="ptr", bufs=2, space="PSUM") as ptr:
            for h in range(HL):
                for tc8 in range(NT):
                    tp = ptr.tile([128, 128], F32, tag="vtp")
                    nc.tensor.transpose(
                        tp, vT_sb[:, h, tc8 * 128 : (tc8 + 1) * 128], ident_sb
                    )
                    nc.scalar.copy(
                        out=v_sb[:, h, OFF // 128 + tc8, :], in_=tp
                    )

        # ---- phase 2: attention, scores^T layout [u, t], both t-slabs per uc ----
        PIPE = 3
        with (
            tc.tile_pool(name="psc", bufs=4, space="PSUM") as psc,
            tc.tile_pool(name="pyt", bufs=2, space="PSUM") as pyt,
            tc.tile_pool(name="pss", bufs=2, space="PSUM") as pss,
        ):
            NU0, NU1 = _nu_of_slab(0), _nu_of_slab(1)
            for h in range(HL):
                ytps = [pyt.tile([128, 512], F32, tag="ytp", name="ytp") for _ in range(NSL)]
                ssps = [pss.tile([128, 512], F32, tag="ssp", name="ssp") for _ in range(NSL)]
                pts = {}

                def slabs_of(uc):
                    return [0, 1] if uc < NU0 else [1]

                def consume(uc):
                    for sl in slabs_of(uc):
                        nu = NU0 if sl == 0 else NU1
                        pt, col = pts[uc]
                        rhs = pt[:, col[sl][0] : col[sl][1]]
                        nc.tensor.matmul(
                            ytps[sl], v_sb[:, h, uc, :], rhs,
                            start=(uc == 0), stop=(uc == nu - 1),
                        )
                        nc.tensor.matmul(
                            ssps[sl], ones_sb, rhs,
                            start=(uc == 0), stop=(uc == nu - 1),
                        )

                for uc in range(NU1):
                    sls = slabs_of(uc)
                    if len(sls) == 2:
                        sbt = sbtp.tile([128, 2, 512], F32, tag="sbt2")
                        pt = ppool.tile([128, 2, 512], BF16, tag="pt2")
                        cols = {0: (0, 512), 1: (512, 1024)}
                    else:
                        sbt = sbtp.tile([128, 1, 512], F32, tag="sbt1")
                        pt = ppool.tile([128, 1, 512], BF16, tag="pt1")
                        cols = {1: (0, 512)}
                    for j, sl in enumerate(sls):
                        scp = psc.tile([128, 512], F32, tag="scp")
                        nc.tensor.matmul(
                            scp,
                            kT_sb[:, h, uc * 128 : (uc + 1) * 128],
                            qT_sb[:, h, sl * 512 : (sl + 1) * 512],
                            start=True, stop=True,
                        )
                        off = sl * 512 + 1408 - uc * 128
                        nc.vector.tensor_add(
                            sbt[:, j, :], scp, wband[:, h, off : off + 512]
                        )
                    nc.scalar.activation(
                        out=pt.rearrange("p a b -> p (a b)"),
                        in_=sbt.rearrange("p a b -> p (a b)"),
                        func=AF.Exp,
                    )
                    pts[uc] = (pt.rearrange("p a b -> p (a b)"), cols)
                    if uc >= PIPE:
                        consume(uc - PIPE)
                for uc in range(max(0, NU1 - PIPE), NU1):
                    consume(uc)

                for sl in range(NSL):
                    ts_ = slice(sl * 512, (sl + 1) * 512)
                    inv = tmp.tile([128, 512], F32, tag="inv")
                    nc.vector.reciprocal(out=inv, in_=ssps[sl])
                    nc.vector.tensor_mul(yT_sb[:, h, ts_], ytps[sl], inv)

        # ---- phase 3: output projection (partial y) ----
        with tc.tile_pool(name="po", bufs=4, space="PSUM") as po:
            for tc8 in range(NT):
                for ns in range(4):
                    pop = po.tile([128, 512], F32, tag="pop")
                    for h in range(HL):
                        nc.tensor.matmul(
                            pop,
                            yT_sb[:, h, tc8 * 128 : (tc8 + 1) * 128],
                            wp_sb[:, h, ns * 512 : (ns + 1) * 512],
                            start=(h == 0),
                            stop=(h == HL - 1),
                        )
                    ystg = ystgp.tile([128, 512], BF16, tag="ystg")
                    nc.scalar.copy(out=ystg, in_=pop)
                    nc.sync.dma_start(
                        out=y_out[
                            tc8 * 128 : (tc8 + 1) * 128, ns * 512 : (ns + 1) * 512
                        ],
                        in_=ystg,
                    )

        # ---- phase 4: new-k output ----
        for h in range(HL):
            nc.sync.dma_start(out=kT_new[h, :, :], in_=kT_sb[:, h, OFF:TF])

    nc.compile()
    return nc


def kernel(x, cached_k, cached_v, W_attn, b_attn, W_proj, b_proj, decay_raw):
    x = np.asarray(x, np.float32)
    cached_k = np.asarray(cached_k, np.float32)
    cached_v = np.asarray(cached_v, np.float32)
    W_attn = np.asarray(W_attn, np.float32)
    b_attn = np.asarray(b_attn, np.float32)
    W_proj = np.asarray(W_proj, np.float32)
    b_proj = np.asarray(b_proj, np.float32)
    decay_raw = np.asarray(decay_raw, np.float32)

    scale = np.float32(1.0 / np.sqrt(HS))
    xT = np.ascontiguousarray(x[0].T).reshape(KC, 128, T).astype(np.float16)

    # RoPE tables (match reference fp32 computation); q tables carry the
    # 1/sqrt(hs) score scale.
    pos = np.arange(OFF, OFF + T).astype(np.float32)
    inv_freq = (
        np.float32(1.0)
        / (np.float32(10000.0) ** (np.arange(HS // 2, dtype=np.float32) / np.float32(HS // 2)))
    ).astype(np.float32)
    ang = (pos[:, None] * inv_freq[None, :]).astype(np.float32)  # [T, 64]
    cos = np.cos(ang).astype(np.float32).T  # [64, T]
    sin = np.sin(ang).astype(np.float32).T
    cos_full = np.concatenate([cos, cos], axis=0)       # [128, T]
    sin_full = np.concatenate([sin, -sin], axis=0)      # [128, T]
    cosf = np.ascontiguousarray(
        np.stack([cos_full * scale, cos_full], axis=1)
    )  # [128, 2, T]
    sinf = np.ascontiguousarray(np.stack([sin_full * scale, sin_full], axis=1))

    decay = np.log1p(np.exp(decay_raw.astype(np.float64))).astype(np.float32)

    ii = np.arange(BW)[None, :] - np.arange(128)[:, None] + CMIN  # d = i - p - 384
    dpos = np.maximum(ii, 0).astype(np.float64)
    logd = np.log1p(dpos)  # [128, BW]

    nc = _build_nc()

    in_maps = []
    for c in range(NCORES):
        g0 = HL * c
        qcols = W_attn[:, g0 * HS : (g0 + HL) * HS]  # scale lives in q rope tables
        kcols = W_attn[:, C + g0 * HS : C + (g0 + HL) * HS]
        vcols = W_attn[:, 2 * C + g0 * HS : 2 * C + (g0 + HL) * HS]
        wqkv_c = (
            np.ascontiguousarray(np.concatenate([qcols, kcols, vcols], axis=1))
            .reshape(KC, 128, 6 * 128)
            .astype(np.float16)
        )

        bq = b_attn[g0 * HS : (g0 + HL) * HS]
        bk = b_attn[C + g0 * HS : C + (g0 + HL) * HS]
        bv = b_attn[2 * C + g0 * HS : 2 * C + (g0 + HL) * HS]
        bqkv_c = np.concatenate([bq, bk, bv]).reshape(6, 128).T.copy()  # [128, 6]

        wp_c = (
            np.ascontiguousarray(W_proj[g0 * HS : (g0 + HL) * HS, :])
            .reshape(HL, 128, C)
            .astype(ml_dtypes.bfloat16)
        )
        ckT_c = np.ascontiguousarray(
            cached_k[0, g0 : g0 + HL].transpose(0, 2, 1)
        ).astype(np.float16)
        cv_c = np.ascontiguousarray(
            cached_v[0, g0 : g0 + HL].reshape(HL, OFF // 128, 128, HS)
        ).astype(ml_dtypes.bfloat16)

        wband_c = np.empty((HL, 128, BW), np.float32)
        for l in range(HL):
            val = -np.log1p(np.float64(decay[g0 + l]) * logd)
            wband_c[l] = np.where(ii >= 0, val, MASKVAL).astype(np.float32)

        in_maps.append(
            {
                "xT": xT,
                "wqkv": wqkv_c,
                "bqkv": bqkv_c,
                "cosf": cosf,
                "sinf": sinf,
                "wp": wp_c,
                "ckT": ckT_c,
                "cv": cv_c,
                "wband_in": wband_c,
            }
        )

    res = run_bass_kernel_spmd(nc, in_maps, core_ids=list(range(NCORES)))
    results = res.results
    kernel._last = results

    # ---- gather ----
    y = np.zeros((T, C), np.float64)
    for c in range(NCORES):
        y += results[c]["y_out"].astype(np.float64)
    y = (y.astype(np.float32) + b_proj[None, :]).reshape(1, T, C)

    k_full = np.empty((1, H, TF, HS), np.float32)
    v_full = np.empty((1, H, TF, HS), np.float32)
    k_full[0, :, :OFF] = cached_k[0]
    v_full[0, :, :OFF] = cached_v[0]
    for c in range(NCORES):
        kT_n = results[c]["kT_new"]   # [HL, 128, T]
        vT_n = results[c]["v_newT"]   # [HL, 128, T]
        for l in range(HL):
            g = HL * c + l
            k_full[0, g, OFF:] = kT_n[l].T
            v_full[0, g, OFF:] = vT_n[l].T

    return (y, k_full, v_full)
